# revision 1
# baseline (speedup 1.0000x reference)
"""Dilated attention Trainium2 kernel.

Problem: B=4, H=16, T=8192, D=64, rates [1,2,3,4].
For rate r: segment S=2^(r+2), dilation dr=2^r; each head h attends causally
within segments l where l % dr == h % dr; output = mean over rates of the
scatter-added per-rate attention outputs.

Strategy (SPMD over 8 cores, 8 (b,h) pairs per core):
  * Host pre-gathers each pair's selected segments per rate into compact
    sequences (total 7680 positions/pair) and pre-transposes Q,K to [64, 7680]
    so the device kernel is gather-free and h-independent (pure SPMD).
  * Device: for each 128-position tile (segments never straddle tiles since
    S | 128): scores_T = K_tile @ Q_tile^T via matmul(lhsT=KT, rhs=QT),
    exp(0.125*s) on ACT, block-diag causal mask multiply on GPSIMD,
    then matmul(lhsT=E_masked, rhs=[V | 4.0]) -> [O_unnorm | 4*denom] with
    denom on the partition axis, DVE reciprocal + per-partition scale.
    (The 4.0 folds the mean over the 4 rates into the normalization.)
  * Host scatter-adds the 4 compact rate outputs back to [T, D].
"""

import sys

import numpy as np

try:
    import concourse.bass as bass  # noqa: F401
except ImportError:
    sys.path.insert(0, "/opt/trn_rl_repo")

import concourse.bass as bass
import concourse.mybir as mybir
import concourse.tile as tile
from concourse import bacc
from concourse.bass_utils import run_bass_kernel_spmd

B, H, T, D = 4, 16, 8192, 64
RATES = [1, 2, 3, 4]
N_CORES = 8
PAIRS_PER_CORE = (B * H) // N_CORES  # 8
TILE_Q = 128

# per-rate constants
SEGS = [2 ** (r + 2) for r in RATES]  # 8, 16, 32, 64
DILS = [2**r for r in RATES]  # 2, 4, 8, 16
TRS = [T // d for d in DILS]  # 4096, 2048, 1024, 512
G_TOTAL = sum(TRS)  # 7680
NTILES = [tr // TILE_Q for tr in TRS]  # 32, 16, 8, 4


def _build_masks() -> np.ndarray:
    """[128, 4*128] fp32; block r holds mask[k, q] for rate r."""
    m = np.zeros((TILE_Q, len(RATES) * TILE_Q), np.float32)
    k = np.arange(TILE_Q)[:, None]
    q = np.arange(TILE_Q)[None, :]
    for ri, s in enumerate(SEGS):
        allowed = (q // s == k // s) & (k % s <= q % s)
        m[:, ri * TILE_Q : (ri + 1) * TILE_Q] = allowed.astype(np.float32)
    return m


def _sel_indices(h: int, r_idx: int) -> np.ndarray:
    s, dr = SEGS[r_idx], DILS[r_idx]
    big_l = T // s
    lp = big_l // dr
    return (h % dr) + np.arange(lp) * dr


def _gather_pair(x: np.ndarray, h: int) -> np.ndarray:
    """x: [T, D] -> compact [7680, D] (concat of per-rate selected segments)."""
    parts = []
    for ri in range(len(RATES)):
        s = SEGS[ri]
        sel = _sel_indices(h, ri)
        parts.append(x.reshape(T // s, s, D)[sel].reshape(-1, D))
    return np.concatenate(parts, axis=0)


def _scatter_pair(og: np.ndarray) -> np.ndarray:
    """og: [7680, D] compact outputs -> scattered/summed [T, D].

    Head-independent scatter is not possible; caller passes per-head og and we
    need h — so this takes (og, h)."""
    raise NotImplementedError


def _scatter_pair_h(og: np.ndarray, h: int) -> np.ndarray:
    out = np.zeros((T, D), np.float32)
    off = 0
    for ri in range(len(RATES)):
        s, tr = SEGS[ri], TRS[ri]
        sel = _sel_indices(h, ri)
        out.reshape(T // s, s, D)[sel] += og[off : off + tr].reshape(-1, s, D)
        off += tr
    return out


def _build_program():
    nc = bacc.Bacc(None, target_bir_lowering=False, debug=False)
    dt = mybir.dt.float32
    qt_d = nc.declare_dram_parameter("qt", [PAIRS_PER_CORE, D, G_TOTAL], dt, isOutput=False)
    kt_d = nc.declare_dram_parameter("kt", [PAIRS_PER_CORE, D, G_TOTAL], dt, isOutput=False)
    v_d = nc.declare_dram_parameter("v", [PAIRS_PER_CORE, G_TOTAL, D], dt, isOutput=False)
    m_d = nc.declare_dram_parameter("mask", [TILE_Q, len(RATES) * TILE_Q], dt, isOutput=False)
    o_d = nc.declare_dram_parameter("o", [PAIRS_PER_CORE, G_TOTAL, D], dt, isOutput=True)

    with tile.TileContext(nc) as tc:
        with (
            tc.tile_pool(name="consts", bufs=1) as consts,
            tc.tile_pool(name="qk", bufs=2) as qk_pool,
            tc.tile_pool(name="vt", bufs=4) as v_pool,
            tc.tile_pool(name="ew", bufs=4) as e_pool,
            tc.tile_pool(name="ot", bufs=4) as o_pool,
            tc.tile_pool(name="ps_s", bufs=4, space="PSUM") as ps_s_pool,
            tc.tile_pool(name="ps_o", bufs=4, space="PSUM") as ps_o_pool,
        ):
            masks = consts.tile([TILE_Q, len(RATES) * TILE_Q], dt)
            nc.sync.dma_start(out=masks[:], in_=m_d[:])

            for p in range(PAIRS_PER_CORE):
                qt_full = qk_pool.tile([D, G_TOTAL], dt, tag="qt")
                kt_full = qk_pool.tile([D, G_TOTAL], dt, tag="kt")
                nc.sync.dma_start(out=qt_full[:], in_=qt_d[p])
                nc.sync.dma_start(out=kt_full[:], in_=kt_d[p])

                off = 0
                for ri in range(len(RATES)):
                    for it in range(NTILES[ri]):
                        t0 = off + it * TILE_Q
                        v1 = v_pool.tile([TILE_Q, D + 1], dt, tag="v1")
                        nc.sync.dma_start(
                            out=v1[:, 0:D], in_=v_d[p, t0 : t0 + TILE_Q, :]
                        )
                        nc.vector.memset(v1[:, D : D + 1], 4.0)

                        ps_s = ps_s_pool.tile([TILE_Q, TILE_Q], dt, tag="ps_s")
                        nc.tensor.matmul(
                            ps_s[:],
                            kt_full[:, t0 : t0 + TILE_Q],
                            qt_full[:, t0 : t0 + TILE_Q],
                            start=True,
                            stop=True,
                        )
                        e = e_pool.tile([TILE_Q, TILE_Q], dt, tag="e")
                        nc.scalar.activation(
                            e[:], ps_s[:], mybir.ActivationFunctionType.Exp, scale=0.125
                        )
                        em = e_pool.tile([TILE_Q, TILE_Q], dt, tag="em")
                        nc.gpsimd.tensor_mul(
                            em[:], e[:], masks[:, ri * TILE_Q : (ri + 1) * TILE_Q]
                        )
                        ps_o = ps_o_pool.tile([TILE_Q, D + 1], dt, tag="ps_o")
                        nc.tensor.matmul(
                            ps_o[:], em[:], v1[:], start=True, stop=True
                        )
                        rec = o_pool.tile([TILE_Q, 1], dt, tag="rec")
                        nc.vector.reciprocal(rec[:], ps_o[:, D : D + 1])
                        o_t = o_pool.tile([TILE_Q, D], dt, tag="o")
                        nc.vector.tensor_scalar_mul(o_t[:], ps_o[:, 0:D], rec[:])
                        nc.sync.dma_start(out=o_d[p, t0 : t0 + TILE_Q, :], in_=o_t[:])
                    off += TRS[ri]
    nc.compile()
    return nc


_PROGRAM_CACHE = {}


def _get_program():
    if "nc" not in _PROGRAM_CACHE:
        _PROGRAM_CACHE["nc"] = _build_program()
    return _PROGRAM_CACHE["nc"]


def prepare_inputs(Q, K, V):
    """Host-side shard+gather+transpose. Returns list of per-core input maps."""
    Q = np.asarray(Q, dtype=np.float32)
    K = np.asarray(K, dtype=np.float32)
    V = np.asarray(V, dtype=np.float32)
    masks = _build_masks()
    in_maps = []
    for c in range(N_CORES):
        qt = np.empty((PAIRS_PER_CORE, D, G_TOTAL), np.float32)
        kt = np.empty((PAIRS_PER_CORE, D, G_TOTAL), np.float32)
        vg = np.empty((PAIRS_PER_CORE, G_TOTAL, D), np.float32)
        for p in range(PAIRS_PER_CORE):
            pair = c * PAIRS_PER_CORE + p
            b, h = divmod(pair, H)
            qt[p] = np.ascontiguousarray(_gather_pair(Q[b, h], h).T)
            kt[p] = np.ascontiguousarray(_gather_pair(K[b, h], h).T)
            vg[p] = _gather_pair(V[b, h], h)
        in_maps.append({"qt": qt, "kt": kt, "v": vg, "mask": masks})
    return in_maps


def finish_outputs(results):
    """results: list of per-core {'o': [8, 7680, 64]} -> full [B, H, T, D]."""
    out = np.zeros((B, H, T, D), np.float32)
    for c in range(N_CORES):
        og = results[c]["o"]
        for p in range(PAIRS_PER_CORE):
            pair = c * PAIRS_PER_CORE + p
            b, h = divmod(pair, H)
            out[b, h] = _scatter_pair_h(og[p], h)
    return out


def kernel(Q, K, V):
    nc = _get_program()
    in_maps = prepare_inputs(Q, K, V)
    res = run_bass_kernel_spmd(nc, in_maps, list(range(N_CORES)))
    return finish_outputs(res.results)


# revision 38
# speedup vs baseline: 25854.2482x; 25854.2482x over previous
"""Dilated attention Trainium2 kernel.

Problem: B=4, H=16, T=8192, D=64, rates [1,2,3,4].
For rate r: segment S=2^(r+2), dilation dr=2^r; each head h attends causally
within segments l where l % dr == h % dr; output = mean over rates of the
scatter-added per-rate attention outputs.

Strategy (SPMD over 8 cores, 8 (b,h) pairs per core):
  * Host pre-gathers each pair's selected segments per rate into compact
    sequences (total 7680 positions = 60 tiles of 128 per pair), and packs the
    transposed Q,K as [128, 3840]: rows 0:64 hold Q^T of even tiles, rows
    64:128 odd tiles, so two K=64 score matmuls run concurrently in the two
    PE-array halves (tile_position row tiling).
  * Device, per supergroup of 8 tiles: 4 concurrent matmul pairs write
    scores_T to a 2-bank PSUM tile (evens bank A, odds bank B), one ACT
    exp(0.125*s) over [128,1024], one mask multiply (block-diag causal), then
    per 4 tiles: 4 PV matmuls (lhsT=E_i, rhs=[V_i | 4.0]) -> [O_i | 4*denom_i]
    in PSUM, one DVE reciprocal over the 4 denom columns, one DVE normalize.
    (The 4.0 folds the mean over the 4 rates into the normalization.)
  * V and O DRAM layouts are [128, 60, *] tile-slot-major (slot order = the
    even/odd permutation within each supergroup); host unpermutes and
    scatter-adds the 4 compact rate outputs back to [T, D].
"""

import contextlib
import sys

import numpy as np

try:
    import concourse.bass as bass  # noqa: F401
except ImportError:
    sys.path.insert(0, "/opt/trn_rl_repo")

import concourse.bass as bass
import concourse.mybir as mybir
import concourse.tile as tile
from concourse import bacc
from concourse.bass_utils import run_bass_kernel_spmd

B, H, T, D = 4, 16, 8192, 64
RATES = [1, 2, 3, 4]
N_CORES = 8
PAIRS_PER_CORE = (B * H) // N_CORES  # 8
TILE_Q = 128
GRP = 4  # tiles per PV/normalize group (PSUM bank limit)
SUPER_GRP = 8  # tiles per scores/exp/mask group
DV = D + 1  # 65: V plus ones column

SEGS = [2 ** (r + 2) for r in RATES]  # 8, 16, 32, 64
DILS = [2**r for r in RATES]  # 2, 4, 8, 16
TRS = [T // d for d in DILS]  # 4096, 2048, 1024, 512
G_TOTAL = sum(TRS)  # 7680
NTILES = [tr // TILE_Q for tr in TRS]  # 32, 16, 8, 4
N_TILES_TOT = G_TOTAL // TILE_Q  # 60
HALF = TILE_Q * N_TILES_TOT // 2  # 3840

MASK_ENG = "split"  # "split" | "dve" | "gpsimd"
LOOKAHEAD = 2  # score supergroups in flight ahead of the consuming tail


def _slot_perm():
    """perm[slot] = global tile index; within each supergroup the evens come
    first, then the odds (matching the PSUM bank split)."""
    perm = []
    off_t = 0
    for ri in range(len(RATES)):
        n = NTILES[ri]
        sg = min(SUPER_GRP, n)
        for g in range(n // sg):
            tg = off_t + g * sg
            perm.extend(tg + 2 * i for i in range(sg // 2))
            perm.extend(tg + 2 * i + 1 for i in range(sg // 2))
        off_t += n
    return np.array(perm)


SLOT_PERM = _slot_perm()


def _build_masks() -> np.ndarray:
    """[128, 4 * 128] fp32; block r holds the per-tile mask[k, q] for rate r."""
    m = np.zeros((TILE_Q, len(RATES) * TILE_Q), np.float32)
    k = np.arange(TILE_Q)[:, None]
    q = np.arange(TILE_Q)[None, :]
    for ri, s in enumerate(SEGS):
        allowed = ((q // s == k // s) & (k % s <= q % s)).astype(np.float32)
        m[:, ri * TILE_Q : (ri + 1) * TILE_Q] = allowed
    return m


def _sel_indices(h: int, r_idx: int) -> np.ndarray:
    s, dr = SEGS[r_idx], DILS[r_idx]
    lp = (T // s) // dr
    return (h % dr) + np.arange(lp) * dr


def _gather_pair(x: np.ndarray, h: int) -> np.ndarray:
    """x: [T, D] -> compact [7680, D] (concat of per-rate selected segments)."""
    parts = []
    for ri in range(len(RATES)):
        s = SEGS[ri]
        sel = _sel_indices(h, ri)
        parts.append(x.reshape(T // s, s, D)[sel].reshape(-1, D))
    return np.concatenate(parts, axis=0)


def _scatter_pair_h(og: np.ndarray, h: int) -> np.ndarray:
    out = np.zeros((T, D), np.float32)
    off = 0
    for ri in range(len(RATES)):
        s, tr = SEGS[ri], TRS[ri]
        sel = _sel_indices(h, ri)
        out.reshape(T // s, s, D)[sel] += og[off : off + tr].reshape(-1, s, D)
        off += tr
    return out


def _bcast_free(ap, count):
    """Repeat a [P, F] AP `count` times along a new middle free dim (step 0)."""
    return bass.AP(tensor=ap.tensor, offset=ap.offset,
                   ap=[ap.ap[0], [0, count], *ap.ap[1:]])


def _mask_engine(nc, gidx):
    if MASK_ENG == "dve":
        return nc.vector
    if MASK_ENG == "gpsimd":
        return nc.gpsimd
    return nc.gpsimd if gidx % 3 != 2 else nc.vector


def _group_list():
    """(ri, j0, sg) for each supergroup of one pair, in slot order."""
    out = []
    off_t = 0
    for ri in range(len(RATES)):
        n = NTILES[ri]
        sg = min(SUPER_GRP, n)
        for g in range(n // sg):
            out.append((ri, off_t + g * sg, sg))
        off_t += n
    return out


GROUPS = _group_list()


def _emit_head(nc, pools, qt_full, kt_full, gr):
    """scores matmuls of one supergroup -> ps_s tile."""
    dt = mybir.dt.float32
    ps_s_pool = pools[4]
    ri, j0, sg = gr
    sh = sg // 2
    m0 = j0 // 2
    # evens -> ps_s cols [0 : sh*128] (bank A), odds -> [512 : ...] (bank B)
    ps_s = ps_s_pool.tile([TILE_Q, 2 * GRP * TILE_Q], dt, tag="ps_s")
    for i in range(sh):
        mc = (m0 + i) * TILE_Q
        nc.tensor.matmul(
            ps_s[:, i * TILE_Q : (i + 1) * TILE_Q],
            kt_full[0:64, mc : mc + TILE_Q],
            qt_full[0:64, mc : mc + TILE_Q],
            start=True,
            stop=True,
            tile_position=(0, 0),
        )
        nc.tensor.matmul(
            ps_s[:, 512 + i * TILE_Q : 512 + (i + 1) * TILE_Q],
            kt_full[64:128, mc : mc + TILE_Q],
            qt_full[64:128, mc : mc + TILE_Q],
            start=True,
            stop=True,
            tile_position=(64, 0),
        )
    return ps_s


STAGES = "all"  # "scores" | "exp" | "mask" | "pv" | "all"


def _emit_tail(nc, pools, ps_s, v_full, o_d, p, masks, gr, gidx, store=True):
    """exp/mask/PV/normalize/store of one supergroup."""
    dt = mybir.dt.float32
    _, _, e_pool, o_pool, _, ps_o_pool = pools
    ri, j0, sg = gr
    sh = sg // 2
    if STAGES == "scores":
        return
    e = e_pool.tile([TILE_Q, sg * TILE_Q], dt, tag="e")
    em = e_pool.tile([TILE_Q, sg * TILE_Q], dt, tag="em")
    mask_sl = masks[:, ri * TILE_Q : (ri + 1) * TILE_Q]
    do_mask = STAGES in ("mask", "pv", "all")
    if sg == SUPER_GRP:
        nc.scalar.activation(
            e[:], ps_s[:], mybir.ActivationFunctionType.Exp, scale=0.125
        )
        if do_mask:
            _mask_engine(nc, gidx).tensor_mul(em[:], e[:], _bcast_free(mask_sl, sg))
    else:
        w = sh * TILE_Q
        for half, c0 in ((0, 0), (1, 512)):
            nc.scalar.activation(
                e[:, half * w : (half + 1) * w],
                ps_s[:, c0 : c0 + w],
                mybir.ActivationFunctionType.Exp,
                scale=0.125,
            )
            if do_mask:
                _mask_engine(nc, gidx).tensor_mul(
                    em[:, half * w : (half + 1) * w],
                    e[:, half * w : (half + 1) * w],
                    _bcast_free(mask_sl, sh),
                )
    if STAGES in ("exp", "mask"):
        return

    o_sg = o_pool.tile([TILE_Q, sg, D], dt, tag="osg")
    for sub in range(sg // GRP):
        ps_o = ps_o_pool.tile([TILE_Q, GRP, DV], dt, tag="ps_o")
        for i in range(GRP):
            s = sub * GRP + i
            nc.tensor.matmul(
                ps_o[:, i, :],
                em[:, s * TILE_Q : (s + 1) * TILE_Q],
                v_full[:, j0 + s, :],
                start=True,
                stop=True,
            )
        if STAGES == "pv":
            continue
        rec = o_pool.tile([TILE_Q, GRP], dt, tag="rec")
        nc.vector.reciprocal(rec[:], ps_o[:, :, D])
        rec_b = bass.AP(
            tensor=rec.tensor,
            offset=rec.offset,
            ap=[rec.ap[0], rec.ap[1], [0, D]],
        )
        nc.vector.tensor_mul(
            o_sg[:, sub * GRP : (sub + 1) * GRP, :], ps_o[:, :, 0:D], rec_b
        )
    if STAGES == "pv":
        return
    if store:
        nc.sync.dma_start(out=o_d[p, :, j0 : j0 + sg, :], in_=o_sg[:])


def _emit_body(nc, pools, qt_d, kt_d, v_d, o_d, masks):
    """Software-pipelined emission: scores of supergroup g+1 are emitted
    before the tail of supergroup g so the PE never waits on exp/mask."""
    dt = mybir.dt.float32
    qk_pool, v_pool = pools[0], pools[1]
    from collections import deque

    gidx = 0
    pending = deque()  # (ps_s, v_full, p, gr)
    for p in range(PAIRS_PER_CORE):
        qt_full = qk_pool.tile([TILE_Q, HALF], dt, tag="qt")
        kt_full = qk_pool.tile([TILE_Q, HALF], dt, tag="kt")
        v_full = v_pool.tile([TILE_Q, N_TILES_TOT, DV], dt, tag="v")
        nc.sync.dma_start(out=qt_full[:], in_=qt_d[p])
        nc.sync.dma_start(out=kt_full[:], in_=kt_d[p])
        nc.sync.dma_start(out=v_full[:], in_=v_d[p])
        for gr in GROUPS:
            ps_s = _emit_head(nc, pools, qt_full, kt_full, gr)
            pending.append((ps_s, v_full, p, gr))
            if len(pending) > LOOKAHEAD:
                pr = pending.popleft()
                _emit_tail(nc, pools, pr[0], pr[1], o_d, pr[2], masks, pr[3], gidx)
                gidx += 1
    while pending:
        pr = pending.popleft()
        _emit_tail(nc, pools, pr[0], pr[1], o_d, pr[2], masks, pr[3], gidx)
        gidx += 1


def _emit_body_dma(nc, pools, qt_d, kt_d, v_d, o_d):
    """Same DRAM traffic as the real body, no compute."""
    dt = mybir.dt.float32
    qk_pool, v_pool = pools[0], pools[1]
    for p in range(PAIRS_PER_CORE):
        qt_full = qk_pool.tile([TILE_Q, HALF], dt, tag="qt")
        kt_full = qk_pool.tile([TILE_Q, HALF], dt, tag="kt")
        v_full = v_pool.tile([TILE_Q, N_TILES_TOT, DV], dt, tag="v")
        nc.sync.dma_start(out=qt_full[:], in_=qt_d[p])
        nc.sync.dma_start(out=kt_full[:], in_=kt_d[p])
        nc.sync.dma_start(out=v_full[:], in_=v_d[p])
        off = 0
        for ri in range(len(RATES)):
            sg = min(SUPER_GRP, NTILES[ri])
            for g in range(NTILES[ri] // sg):
                j0 = off // TILE_Q + g * sg
                nc.sync.dma_start(
                    out=o_d[p, :, j0 : j0 + sg, :],
                    in_=v_full[:, j0 : j0 + sg, 0:D],
                )
            off += TRS[ri]


def _emit_body_compute(nc, pools, qt_d, kt_d, v_d, o_d, masks):
    """Full compute on SBUF-resident data for one pair, repeated 8x."""
    dt = mybir.dt.float32
    qk_pool, v_pool = pools[0], pools[1]
    qt_full = qk_pool.tile([TILE_Q, HALF], dt, tag="qt")
    kt_full = qk_pool.tile([TILE_Q, HALF], dt, tag="kt")
    v_full = v_pool.tile([TILE_Q, N_TILES_TOT, DV], dt, tag="v")
    nc.sync.dma_start(out=qt_full[:], in_=qt_d[0])
    nc.sync.dma_start(out=kt_full[:], in_=kt_d[0])
    nc.sync.dma_start(out=v_full[:], in_=v_d[0])
    gidx = 0
    prev = None
    for p in range(PAIRS_PER_CORE):
        for gr in GROUPS:
            ps_s = _emit_head(nc, pools, qt_full, kt_full, gr)
            if prev is not None:
                _emit_tail(nc, pools, prev[0], v_full, o_d, 0, masks, prev[1],
                           gidx, store=False)
                gidx += 1
            prev = (ps_s, gr)
    _emit_tail(nc, pools, prev[0], v_full, o_d, 0, masks, prev[1], gidx, store=True)


def _build_program(body_reps: int = 1, variant: str = "full"):
    nc = bacc.Bacc(None, target_bir_lowering=False, debug=False)
    dt = mybir.dt.float32
    qt_d = nc.declare_dram_parameter("qt", [PAIRS_PER_CORE, TILE_Q, HALF], dt, isOutput=False)
    kt_d = nc.declare_dram_parameter("kt", [PAIRS_PER_CORE, TILE_Q, HALF], dt, isOutput=False)
    v_d = nc.declare_dram_parameter(
        "v", [PAIRS_PER_CORE, TILE_Q, N_TILES_TOT, DV], dt, isOutput=False
    )
    m_d = nc.declare_dram_parameter(
        "mask", [TILE_Q, len(RATES) * TILE_Q], dt, isOutput=False
    )
    o_d = nc.declare_dram_parameter(
        "o", [PAIRS_PER_CORE, TILE_Q, N_TILES_TOT, D], dt, isOutput=True
    )

    with tile.TileContext(nc) as tc:
        with (
            tc.tile_pool(name="consts", bufs=1) as consts,
            tc.tile_pool(name="qk", bufs=2) as qk_pool,
            tc.tile_pool(name="vt", bufs=2) as v_pool,
            tc.tile_pool(name="ew", bufs=6) as e_pool,
            tc.tile_pool(name="ot", bufs=6) as o_pool,
            tc.tile_pool(name="ps_s", bufs=3, space="PSUM") as ps_s_pool,
            tc.tile_pool(name="ps_o", bufs=2, space="PSUM") as ps_o_pool,
        ):
            masks = consts.tile([TILE_Q, len(RATES) * TILE_Q], dt)
            nc.sync.dma_start(out=masks[:], in_=m_d[:])
            pools = (qk_pool, v_pool, e_pool, o_pool, ps_s_pool, ps_o_pool)

            loop_ctx = (
                tc.For_i(0, body_reps, 1) if body_reps > 1 else contextlib.nullcontext()
            )
            with loop_ctx:
                if variant == "full":
                    _emit_body(nc, pools, qt_d, kt_d, v_d, o_d, masks)
                elif variant == "dma":
                    _emit_body_dma(nc, pools, qt_d, kt_d, v_d, o_d)
                elif variant == "compute":
                    _emit_body_compute(nc, pools, qt_d, kt_d, v_d, o_d, masks)
    nc.compile()
    return nc


_PROGRAM_CACHE = {}


def _get_program():
    if "nc" not in _PROGRAM_CACHE:
        _PROGRAM_CACHE["nc"] = _build_program()
    return _PROGRAM_CACHE["nc"]


def prepare_inputs(Q, K, V):
    """Host-side shard+gather+transpose+pack. Returns per-core input maps."""
    Q = np.asarray(Q, dtype=np.float32)
    K = np.asarray(K, dtype=np.float32)
    V = np.asarray(V, dtype=np.float32)
    masks = _build_masks()
    in_maps = []
    for c in range(N_CORES):
        qt = np.empty((PAIRS_PER_CORE, TILE_Q, HALF), np.float32)
        kt = np.empty((PAIRS_PER_CORE, TILE_Q, HALF), np.float32)
        vg = np.empty((PAIRS_PER_CORE, TILE_Q, N_TILES_TOT, DV), np.float32)
        vg[:, :, :, D] = 4.0
        for p in range(PAIRS_PER_CORE):
            pair = c * PAIRS_PER_CORE + p
            b, h = divmod(pair, H)
            qg = _gather_pair(Q[b, h], h).T  # [64, 7680]
            kg = _gather_pair(K[b, h], h).T
            # pack: col-block m rows 0:64 = tile 2m, rows 64:128 = tile 2m+1
            qt[p, 0:64] = qg.reshape(D, N_TILES_TOT, TILE_Q)[:, 0::2].reshape(D, HALF)
            qt[p, 64:128] = qg.reshape(D, N_TILES_TOT, TILE_Q)[:, 1::2].reshape(D, HALF)
            kt[p, 0:64] = kg.reshape(D, N_TILES_TOT, TILE_Q)[:, 0::2].reshape(D, HALF)
            kt[p, 64:128] = kg.reshape(D, N_TILES_TOT, TILE_Q)[:, 1::2].reshape(D, HALF)
            vp = _gather_pair(V[b, h], h)  # [7680, 64]
            vt = vp.reshape(N_TILES_TOT, TILE_Q, D)[SLOT_PERM]  # slot order
            vg[p, :, :, 0:D] = vt.transpose(1, 0, 2)
        in_maps.append({"qt": qt, "kt": kt, "v": vg, "mask": masks})
    return in_maps


def finish_outputs(results):
    """results: list of per-core {'o': [8, 128, 60, 64]} -> full [B, H, T, D]."""
    inv = np.argsort(SLOT_PERM)
    out = np.zeros((B, H, T, D), np.float32)
    for c in range(N_CORES):
        og = results[c]["o"]  # [8, 128, 60, 64] slot-major
        for p in range(PAIRS_PER_CORE):
            pair = c * PAIRS_PER_CORE + p
            b, h = divmod(pair, H)
            tiles = og[p].transpose(1, 0, 2)  # [60(slots), 128, 64]
            compact = tiles[inv].reshape(G_TOTAL, D)
            out[b, h] = _scatter_pair_h(compact, h)
    return out


def kernel(Q, K, V):
    nc = _get_program()
    in_maps = prepare_inputs(Q, K, V)
    res = run_bass_kernel_spmd(nc, in_maps, list(range(N_CORES)))
    return finish_outputs(res.results)
